# revision 32
# baseline (speedup 1.0000x reference)
"""Trainium2 Bass kernel for nn_Attention_25580825215518 (linear attention
with L2-normalized q/k over sequence, LePE depthwise 3x3 conv, qkv+proj).

Sharding: data-parallel over batch B=8, one batch element per NeuronCore.
No collectives needed; host does the (free) layout transposes.

Per-core device pipeline (everything feature-on-partition, "transposed"):
  xT [384,4096]f32 --(f32r matmuls, W stationary)--> qkvT in PSUM
    -> evict bf16: qT, kT (dense), vT (dense) [+bias]
  vT dense --DMA strided copy--> vpad [128, 66*66] (zero border)
  kT/vT --DMA xbar transpose--> kN/vN natural [l,c] chunks for G = k^T v
  norms: ACT Square+accum over L -> 1/||q_c||, 1/||k_c||  (fold into attn scales)
  G (PSUM, 3x [128,384], f32) = sum_l kN^T vN   (full 384x384; only blockdiag used)
  attn = temp * diag(1/nk) G   (per-partition scale on evict)  -> DMA out
  A2   = diag(1/nq) attn (bf16, block-diag zeroed)
  outT = A2^T @ qT (bf16 matmuls) + lepe (DVE fused MAC evict)
  lepe = depthwise 3x3 conv on vpad/vT via 9 per-partition-scalar MACs (DVE)
  projT = proj_wT^T @ outT (bf16) + bias -> DMA out [384,4096] f32
Host: out[b] = projT.T; attn assembled directly.
"""

import numpy as np
import ml_dtypes
from contextlib import ExitStack

import concourse.bass as bass
import concourse.tile as tile
from concourse import bacc, mybir
from concourse.bass_utils import run_bass_kernel_spmd

N_CORES = 8
B, L, C = 8, 4096, 384
NH, HD = 8, 48          # heads, head dim
IMG = 64                # H = W = 64
PADW = 66               # padded image row width
CT = 3                  # 128-row tiles covering C=384
SL = 8                  # 512-wide l slices
NT = 32                 # 128-wide l chunks

f32 = mybir.dt.float32
f32r = mybir.dt.float32r
bf16 = mybir.dt.bfloat16


def _head_segments():
    """(mk, r0, rr, h, d0): head-diagonal blocks split at 128-row tile bounds."""
    segs = []
    for h in range(NH):
        lo, hi = h * HD, (h + 1) * HD
        for mk in range(CT):
            a, b = max(lo, mk * 128), min(hi, (mk + 1) * 128)
            if a < b:
                segs.append((mk, a - mk * 128, b - a, h, a - lo))
    return segs


def build_kernel(dbg=False):
    AF = mybir.ActivationFunctionType
    OP = mybir.AluOpType

    nc = bacc.Bacc("TRN2", target_bir_lowering=False, debug=False,
                   num_devices=N_CORES)
    dbg_d = {}
    if dbg:
        for nm, shp in [("d_qT", [128, L]), ("d_kT", [128, L]),
                        ("d_vd", [128, L]), ("d_lepe", [128, L]),
                        ("d_kn0", [128, C]), ("d_vn0", [128, C]),
                        ("d_asb", [CT * 128, C]), ("d_a2", [CT * 128, C]),
                        ("d_nq", [CT * 128, 1]), ("d_nk", [CT * 128, 1]),
                        ("d_outT", [128, L])]:
            dt = f32 if nm in ("d_asb", "d_nq", "d_nk") else bf16
            dbg_d[nm] = nc.dram_tensor(nm, shp, dt, kind="ExternalOutput").ap()

    xT_d = nc.dram_tensor("xT", [C, L], f32, kind="ExternalInput").ap()
    qkvwT_d = nc.dram_tensor("qkvwT", [C, 3 * C], f32, kind="ExternalInput").ap()
    qkvb_d = nc.dram_tensor("qkvb", [3 * C], f32, kind="ExternalInput").ap()
    pwT_d = nc.dram_tensor("projwT", [C, C], bf16, kind="ExternalInput").ap()
    pb_d = nc.dram_tensor("projb", [C], f32, kind="ExternalInput").ap()
    temp_d = nc.dram_tensor("tempc", [C], f32, kind="ExternalInput").ap()
    cw_d = nc.dram_tensor("convw", [C, 9], f32, kind="ExternalInput").ap()
    cb_d = nc.dram_tensor("convb", [C], f32, kind="ExternalInput").ap()
    bdm_d = nc.dram_tensor("bdmask", [C, C], bf16, kind="ExternalInput").ap()
    id_d = nc.dram_tensor("ident", [128, 128], bf16, kind="ExternalInput").ap()
    out_d = nc.dram_tensor("projT", [C, L], f32, kind="ExternalOutput").ap()
    attn_d = nc.dram_tensor("attn", [NH, HD, HD], f32, kind="ExternalOutput").ap()

    with tile.TileContext(nc) as tc, ExitStack() as ctx:
        P = lambda **kw: ctx.enter_context(tc.tile_pool(**kw))
        xp = P(name="xp", bufs=3)            # streamed xT slices, f32r
        wq = P(name="wq", bufs=3)            # qkv_wT, f32r
        wp = P(name="wp", bufs=3)            # proj_wT, bf16
        qk = P(name="qk", bufs=6)            # qT(3)+kT(3) / outT(3) bf16 [128,4096]
        vd = P(name="vd", bufs=3)            # v dense bf16 [128,4096]
        vp = P(name="vp", bufs=3)            # v padded bf16 [128,66*66]
        lp = P(name="lp", bufs=3)            # lepe bf16 [128,4096]
        tkn = P(name="tkn", bufs=22)         # kN quad tiles [128,512] bf16
        tvn = P(name="tvn", bufs=22)         # vN quad tiles [128,512] bf16
        a2p = P(name="a2p", bufs=6)          # A2 + blockdiag mask [128,384] bf16
        asb = P(name="asb", bufs=3)          # attn staging [128,384] f32
        fin = P(name="fin", bufs=2)          # final [128,512] f32
        nsc = P(name="nsc", bufs=1)          # norm scratch [128,512] bf16
        sm = P(name="sm", bufs=28)           # [128,small] bits
        ps = P(name="ps", bufs=5, space="PSUM")
        pg = P(name="pg", bufs=3, space="PSUM")

        # ---------- constants ----------
        wq_t = []
        for kc in range(CT):
            t = wq.tile([128, 3 * C], f32r, tag="wq", name="wqt")
            nc.sync.dma_start(t[:], qkvwT_d[kc * 128:(kc + 1) * 128, :].bitcast(f32r))
            wq_t.append(t)
        wp_t = []
        for kc in range(CT):
            t = wp.tile([128, C], bf16, tag="wp", name="wpt")
            nc.sync.dma_start(t[:], pwT_d[kc * 128:(kc + 1) * 128, :])
            wp_t.append(t)
        qkvb_t = []
        for m in range(9):
            t = sm.tile([128, 1], f32, tag="sm", name="qkvbt")
            nc.sync.dma_start(t[:], qkvb_d[m * 128:(m + 1) * 128].unsqueeze(1))
            qkvb_t.append(t)

        def col_loads(src):
            out = []
            for kc in range(CT):
                t = sm.tile([128, 1], f32, tag="sm", name="colt")
                nc.sync.dma_start(t[:], src[kc * 128:(kc + 1) * 128].unsqueeze(1))
                out.append(t)
            return out

        pb_t = col_loads(pb_d)
        temp_t = col_loads(temp_d)
        cb_t = col_loads(cb_d)
        cw_t = []
        for kc in range(CT):
            t = sm.tile([128, 9], f32, tag="smw", name="cwt", bufs=3)
            nc.sync.dma_start(t[:], cw_d[kc * 128:(kc + 1) * 128, :])
            cw_t.append(t)

        bdm_t = []
        for kc in range(CT):
            t = a2p.tile([128, C], bf16, tag="bdm", name="bdmt")
            nc.sync.dma_start(t[:], bdm_d[kc * 128:(kc + 1) * 128, :])
            bdm_t.append(t)
        ident_t = sm.tile([128, 128], bf16, tag="ident", name="identt", bufs=1)
        nc.sync.dma_start(ident_t[:], id_d[:])

        vpad_t = []
        for kc in range(CT):
            t = vp.tile([128, PADW * PADW], bf16, tag="vp", name="vpadt")
            nc.gpsimd.memset(t[:], 0.0)
            vpad_t.append(t)
        a2_t = [a2p.tile([128, C], bf16, tag="a2", name="a2t")
                for _ in range(CT)]

        # ---------- phase 1: qkv projection, group order v, k, q ----------
        # m index: q rows 0..383 -> m 0..2, k -> 3..5, v -> 6..8
        qT_t = [qk.tile([128, L], bf16, tag="qk", name="qTt") for _ in range(CT)]
        kT_t = [qk.tile([128, L], bf16, tag="qk", name="kTt") for _ in range(CT)]
        vd_t = [vd.tile([128, L], bf16, tag="vd", name="vdt") for _ in range(CT)]

        def qkv_group(ms, dest):
            for sl in range(SL):
                xt = []
                for kc in range(CT):
                    t = xp.tile([128, 512], f32r, tag="xp", name="xt")
                    nc.sync.dma_start(
                        t[:], xT_d[kc * 128:(kc + 1) * 128,
                                   sl * 512:(sl + 1) * 512].bitcast(f32r))
                    xt.append(t)
                for mi, m in enumerate(ms):
                    pst = ps.tile([128, 512], f32, tag="ps", name="pst")
                    for kc in range(CT):
                        nc.tensor.matmul(
                            pst[:], wq_t[kc][:, m * 128:(m + 1) * 128], xt[kc][:],
                            start=(kc == 0), stop=(kc == CT - 1))
                    # evict with bias on ACT -> bf16 dense
                    nc.scalar.activation(
                        dest[mi][:, sl * 512:(sl + 1) * 512], pst[:],
                        AF.Identity, bias=qkvb_t[m][:, 0:1])

        qkv_group([6, 7, 8], vd_t)

        # vpad strided build (DMA sbuf->sbuf) + v transposes
        for kc in range(CT):
            dense_v = vd_t[kc][:].rearrange("p (y x) -> p y x", y=IMG)
            padv = vpad_t[kc][:].rearrange("p (y x) -> p y x", y=PADW)
            nc.sync.dma_start(padv[:, 1:1 + IMG, 1:1 + IMG], dense_v[:, :, :])
        # PE-transpose v -> natural layout, 4 chunks packed per PSUM bank.
        # Only the first transpose into a bank sets start=True (bank clear);
        # the rest overwrite their (cleared) column slice.
        def pe_transpose_quads(src_t, pool, tag, evict_engine):
            out = []   # out[kc][g] = [128, 512] bf16, chunk t at cols (t%4)*128
            for kc in range(CT):
                groups = []
                for g in range(NT // 4):
                    trp = ps.tile([128, 512], bf16, tag="ps", name="trp")
                    for i in range(4):
                        t = g * 4 + i
                        nc.tensor.matmul(
                            trp[:, i * 128:(i + 1) * 128],
                            src_t[kc][:, t * 128:(t + 1) * 128], ident_t[:],
                            start=(i == 0), stop=(i == 3),
                            is_transpose=True, skip_group_check=True)
                    q4 = pool.tile([128, 512], bf16, tag=tag, name=tag + "t")
                    if evict_engine == "vector":
                        nc.vector.tensor_copy(q4[:], trp[:])
                    else:
                        nc.scalar.activation(q4[:], trp[:], AF.Copy)
                    groups.append(q4)
                out.append(groups)
            return out

        vN4 = pe_transpose_quads(vd_t, tvn, "vn", "vector")

        # conv lepe: tap0 on ACT (init with conv bias), rest DVE MACs.
        lepe_t = [lp.tile([128, L], bf16, tag="lp", name="lept") for _ in range(CT)]

        def emit_conv(kc):
            padv = vpad_t[kc][:].rearrange("p (y x) -> p y x", y=PADW)
            dense_v = vd_t[kc][:].rearrange("p (y x) -> p y x", y=IMG)
            lep = lepe_t[kc][:].rearrange("p (y x) -> p y x", y=IMG)
            nc.scalar.activation(
                lep[:, :, :], padv[:, 0:IMG, 0:IMG], AF.Identity,
                bias=cb_t[kc][:, 0:1], scale=cw_t[kc][:, 0:1])
            for t in range(1, 9):
                dyi, dxi = divmod(t, 3)
                if dxi == 1:
                    dy = dyi - 1
                    oy0, iy0, yc = (1, 0, IMG - 1) if dy < 0 else (
                        (0, 0, IMG) if dy == 0 else (0, 1, IMG - 1))
                    nc.vector.scalar_tensor_tensor(
                        lep[:, oy0:oy0 + yc, :], dense_v[:, iy0:iy0 + yc, :],
                        cw_t[kc][:, t:t + 1], lep[:, oy0:oy0 + yc, :],
                        OP.mult, OP.add)
                else:
                    nc.vector.scalar_tensor_tensor(
                        lep[:, :, :], padv[:, dyi:dyi + IMG, dxi:dxi + IMG],
                        cw_t[kc][:, t:t + 1], lep[:, :, :],
                        OP.mult, OP.add)

        emit_conv(0)
        emit_conv(1)

        qkv_group([3, 4, 5], kT_t)

        kN4 = pe_transpose_quads(kT_t, tkn, "kn", "scalar")

        # k norms on ACT (squares in 512-col chunks, accumulate partials)
        def emit_norm(src_t):
            parts, norms = [], []
            for kc in range(CT):
                part = sm.tile([128, SL], f32, tag="smp", name="partt", bufs=12)
                for sl in range(SL):
                    scr = nsc.tile([128, 512], bf16, tag="nsc", name="scrt")
                    nc.scalar.activation(
                        scr[:], src_t[kc][:, sl * 512:(sl + 1) * 512],
                        AF.Square, accum_out=part[:, sl:sl + 1])
                parts.append(part)
            for kc in range(CT):
                dummy = sm.tile([128, SL], f32, tag="smp", name="dumt", bufs=12)
                n2 = sm.tile([128, 1], f32, tag="sm", name="n2t")
                nc.scalar.activation(dummy[:], parts[kc][:], AF.Copy,
                                     accum_out=n2[:, 0:1])
                n1 = sm.tile([128, 1], f32, tag="sm", name="n1t")
                nc.scalar.sqrt(n1[:], n2[:])
                norms.append(n1)
            return norms

        nk_t = emit_norm(kT_t)

        qkv_group([0, 1, 2], qT_t)
        nq_t = emit_norm(qT_t)

        # DVE small chain: reciprocals + scales (emitted after conv kc=1)
        kscale_t, rq_t = [], []
        for kc in range(CT):
            rk = sm.tile([128, 1], f32, tag="sm", name="rkt")
            nc.vector.reciprocal(rk[:], nk_t[kc][:])
            ksc = sm.tile([128, 1], f32, tag="sm", name="ksct")
            nc.vector.tensor_mul(ksc[:], rk[:], temp_t[kc][:])
            kscale_t.append(ksc)
        for kc in range(CT):
            rq = sm.tile([128, 1], f32, tag="sm", name="rqt")
            nc.vector.reciprocal(rq[:], nq_t[kc][:])
            rq_t.append(rq)

        emit_conv(2)

        # ---------- phase 2: G = k^T v (full 384x384, accumulate over l) ----------
        # NOTE: start=True clears the whole PSUM bank, so only the very first
        # matmul touching each g_ps tile may set it; later column-slice groups
        # overwrite (has_written=0) then accumulate.
        g_ps = [pg.tile([128, C], f32, tag="pg", name="gps") for _ in range(CT)]
        for t in range(NT):
            g, i = divmod(t, 4)
            sl128 = slice(i * 128, (i + 1) * 128)
            for mk in range(CT):
                for vc in range(CT):
                    nc.tensor.matmul(
                        g_ps[mk][:, vc * 128:(vc + 1) * 128],
                        kN4[mk][g][:, sl128], vN4[vc][g][:, sl128],
                        start=(t == 0 and vc == 0),
                        stop=(t == NT - 1 and vc == CT - 1),
                        skip_group_check=True)

        # evict G full-tile: attn staging = G * (temp/nk) on ACT; then
        # A2 = asb * (1/nq) * blockdiag_mask as one fused STT on gpsimd.
        asb_t = [asb.tile([128, C], f32, tag="asb", name="asbt") for _ in range(CT)]
        for mk in range(CT):
            nc.scalar.activation(asb_t[mk][:], g_ps[mk][:], AF.Copy,
                                 scale=kscale_t[mk][:, 0:1])
        segs = _head_segments()
        for mk in range(CT):
            nc.vector.scalar_tensor_tensor(
                a2_t[mk][:], asb_t[mk][:], rq_t[mk][:, 0:1], bdm_t[mk][:],
                OP.mult, OP.mult)
        for (mk, r0, rr, h, d0) in segs:
            nc.sync.dma_start(attn_d[h, d0:d0 + rr, :],
                              asb_t[mk][r0:r0 + rr, h * HD:(h + 1) * HD])

        # ---------- phase 3+4: outT = A2^T qT + lepe; projT = W^T outT + b ----
        NONZERO = {0: (0, 1), 1: (0, 1, 2), 2: (1, 2)}
        outT_t = [qk.tile([128, L], bf16, tag="qk", name="outTt") for _ in range(CT)]

        def out_mm(sl):
            psts = []
            for mo in range(CT):
                pst = ps.tile([128, 512], f32, tag="ps", name="opst")
                kcs = NONZERO[mo]
                for i, kc in enumerate(kcs):
                    nc.tensor.matmul(
                        pst[:], a2_t[kc][:, mo * 128:(mo + 1) * 128],
                        qT_t[kc][:, sl * 512:(sl + 1) * 512],
                        start=(i == 0), stop=(i == len(kcs) - 1))
                psts.append(pst)
            return psts

        def out_evict(sl, psts):
            for mo in range(CT):
                nc.vector.scalar_tensor_tensor(
                    outT_t[mo][:, sl * 512:(sl + 1) * 512], psts[mo][:],
                    1.0, lepe_t[mo][:, sl * 512:(sl + 1) * 512],
                    OP.mult, OP.add)

        def proj_mm(sl):
            for mo in range(CT):
                pst = ps.tile([128, 512], f32, tag="ps", name="ppst")
                for kc in range(CT):
                    nc.tensor.matmul(
                        pst[:], wp_t[kc][:, mo * 128:(mo + 1) * 128],
                        outT_t[kc][:, sl * 512:(sl + 1) * 512],
                        start=(kc == 0), stop=(kc == CT - 1))
                f = fin.tile([128, 512], f32, tag="fin", name="fint")
                nc.scalar.activation(f[:], pst[:], AF.Identity,
                                     bias=pb_t[mo][:, 0:1])
                nc.sync.dma_start(
                    out_d[mo * 128:(mo + 1) * 128, sl * 512:(sl + 1) * 512], f[:])

        pend = []
        for sl in range(SL):
            pend.append((sl, out_mm(sl)))
            if sl >= 1:
                psl, psts = pend.pop(0)
                out_evict(psl, psts)
                proj_mm(psl)
        psl, psts = pend.pop(0)
        out_evict(psl, psts)
        proj_mm(psl)

        if dbg:
            nc.sync.dma_start(dbg_d["d_qT"][:], qT_t[0][:])
            nc.sync.dma_start(dbg_d["d_kT"][:], kT_t[0][:])
            nc.sync.dma_start(dbg_d["d_vd"][:], vd_t[0][:])
            nc.sync.dma_start(dbg_d["d_lepe"][:], lepe_t[0][:])
            for kc in range(CT):
                nc.sync.dma_start(dbg_d["d_kn0"][:, kc * 128:(kc + 1) * 128],
                                  kN4[kc][0][:, 0:128])
                nc.sync.dma_start(dbg_d["d_vn0"][:, kc * 128:(kc + 1) * 128],
                                  vN4[kc][0][:, 0:128])
            nc.sync.dma_start(dbg_d["d_outT"][:], outT_t[0][:])
            for kc in range(CT):
                nc.sync.dma_start(
                    dbg_d["d_asb"][kc * 128:(kc + 1) * 128, :], asb_t[kc][:])
                nc.sync.dma_start(
                    dbg_d["d_a2"][kc * 128:(kc + 1) * 128, :], a2_t[kc][:])
                nc.sync.dma_start(
                    dbg_d["d_nq"][kc * 128:(kc + 1) * 128, :], nq_t[kc][:])
                nc.sync.dma_start(
                    dbg_d["d_nk"][kc * 128:(kc + 1) * 128, :], nk_t[kc][:])

    nc.compile()
    return nc


_NC = None


def _get_nc():
    global _NC
    if _NC is None:
        _NC = build_kernel()
    return _NC


def make_in_maps(x, qkv_w, qkv_b, proj_w, proj_b, temperature, conv_w, conv_b):
    x = np.asarray(x, dtype=np.float32)
    qkvwT = np.ascontiguousarray(np.asarray(qkv_w, dtype=np.float32).T)
    qkvb = np.ascontiguousarray(np.asarray(qkv_b, dtype=np.float32))
    projwT = np.ascontiguousarray(
        np.asarray(proj_w, dtype=np.float32).T).astype(ml_dtypes.bfloat16)
    projb = np.ascontiguousarray(np.asarray(proj_b, dtype=np.float32))
    tempc = np.repeat(np.asarray(temperature, dtype=np.float32).reshape(NH), HD)
    tempc = np.ascontiguousarray(tempc)
    convw = np.ascontiguousarray(
        np.asarray(conv_w, dtype=np.float32).reshape(C, 9))
    convb = np.ascontiguousarray(np.asarray(conv_b, dtype=np.float32))
    ident = np.eye(128, dtype=ml_dtypes.bfloat16)
    bdmask = np.zeros((C, C), dtype=ml_dtypes.bfloat16)
    for h in range(NH):
        bdmask[h * HD:(h + 1) * HD, h * HD:(h + 1) * HD] = 1
    in_maps = []
    for b in range(N_CORES):
        in_maps.append({
            "xT": np.ascontiguousarray(x[b].T), "ident": ident,
            "qkvwT": qkvwT, "qkvb": qkvb, "projwT": projwT, "projb": projb,
            "tempc": tempc, "convw": convw, "convb": convb, "bdmask": bdmask,
        })
    return in_maps


def assemble(results):
    out = np.empty((B, L, C), dtype=np.float32)
    attn = np.empty((B, NH, HD, HD), dtype=np.float32)
    for b in range(N_CORES):
        out[b] = results[b]["projT"].T
        attn[b] = results[b]["attn"]
    return out, attn


def kernel(x, qkv_w, qkv_b, proj_w, proj_b, temperature, conv_w, conv_b):
    nc = _get_nc()
    in_maps = make_in_maps(x, qkv_w, qkv_b, proj_w, proj_b, temperature,
                           conv_w, conv_b)
    res = run_bass_kernel_spmd(nc, in_maps, core_ids=list(range(N_CORES)))
    return assemble(res.results)
